# revision 4
# baseline (speedup 1.0000x reference)
"""Additive attention kernel for 8 Trainium2 NeuronCores (v2).

Math: scores[b,i,j] = sum_d tanh(q[b,i,d] + k[b,j,d]); out = softmax_j(scores) @ v.
tanh(s) ~= sum_m C[m] sin(W[m] s) (M=8, bf16-exact W/2pi) and
sin(w(q+k)) = sin(wq)cos(wk) + cos(wq)sin(wk) -> rank-1024 bf16 PE matmul.

Angle path in turns (t = w x / 2pi):
  t0_psum = diag(w/2pi) @ (x_hi + x_lo)        (PE; host splits x = hi+lo bf16)
  n       = (t0 + MAGIC) - MAGIC               (DVE magic round; cos: +0.25 shifted)
  red    += (-I) @ n                           (PE accumulate)
  feat    = Sin(2pi * red) -> bf16             (ScalarE, 1024-wide merged acts)

v2 scheduling: PE-clock warm-up dummies during DMA wait; Sin/Exp act tables
hoisted off the critical path via dummy activations; inputs DMA'd on three
queues in parallel; K-side acts merged 1024-wide; tail pipelined per output
chunk with out-DMAs split across queues.

Sharding: B=8 -> one batch per core, no collectives.
"""

import math

import numpy as np
import ml_dtypes

import concourse.bass as bass
import concourse.mybir as mybir
from concourse.bass_utils import run_bass_kernel_spmd

F32 = mybir.dt.float32
BF16 = mybir.dt.bfloat16
AF = mybir.ActivationFunctionType
ALU = mybir.AluOpType

W0 = [0.273822509, 0.825679394, 1.38832881, 1.96485759,
      2.55624192, 3.16272728, 3.77941797, 4.47596827]

B, L, D, M = 8, 512, 64, 8
TWO_PI = 2.0 * math.pi
MAGIC = 12582912.0  # 1.5 * 2^23

PAIRS = [(0, 1), (2, 3), (4, 5), (6, 7)]


def _bf(x):
    return np.asarray(x).astype(ml_dtypes.bfloat16)


def _fit_consts():
    w2pi = _bf(np.array(W0, np.float32) / TWO_PI).astype(np.float64)
    w_eff = w2pi * TWO_PI
    S = 9.8
    sg = np.linspace(-S, S, 4001)
    wts = np.exp(-(sg**2) / 4) + 0.02
    A = np.sin(np.outer(sg, w_eff)) * np.sqrt(wts)[:, None]
    c, *_ = np.linalg.lstsq(A, np.tanh(sg) * np.sqrt(wts), rcond=None)
    return w2pi.astype(np.float32), c.astype(np.float32)


W2PI, C = _fit_consts()

_CACHE = {}


def _build():
    nc = bass.Bass()
    qhl_ext = nc.declare_dram_parameter("qhl", [128, L], BF16, isOutput=False)
    khl_ext = nc.declare_dram_parameter("khl", [128, L], BF16, isOutput=False)
    vh_ext = nc.declare_dram_parameter("vh", [L, 65], BF16, isOutput=False)
    dg_ext = nc.declare_dram_parameter("dg", [128, 9, 128], BF16, isOutput=False)
    amp_ext = nc.declare_dram_parameter("amp", [128, 4], F32, isOutput=False)
    out_ext = nc.declare_dram_parameter("out", [L, D], F32, isOutput=True)

    from contextlib import ExitStack

    with ExitStack() as ctx:
        e = ctx.enter_context
        QHL = e(nc.sbuf_tensor("QHL", [128, L], BF16))
        KHL = e(nc.sbuf_tensor("KHL", [128, L], BF16))
        # DG slots: sin_j=2j, cos_j=2j+1 (row 127 = 0.25 ones -> t+0.25), 8: -I
        DG = e(nc.sbuf_tensor([128, 9, 128], BF16))
        AMP = e(nc.sbuf_tensor([128, 4], F32))
        VH = e(nc.sbuf_tensor([128, 4, 65], BF16))
        # per-unit round output: [unit, bank-within-slot-pair, L]
        NR = e(nc.sbuf_tensor("NR", [128, 8, 2, L], BF16))
        FQRAW = e(nc.sbuf_tensor([128, 4, 2, L], BF16))
        FQS = e(nc.sbuf_tensor([128, 4, 2, L], BF16))
        FK = e(nc.sbuf_tensor([128, 4, 2, L], BF16))
        EXPT = e(nc.sbuf_tensor([128, 4, L], BF16))
        RCP = e(nc.sbuf_tensor([128, 4], F32))
        OUT = e(nc.sbuf_tensor([128, 4, D], F32))
        DUMW = e(nc.sbuf_tensor([128, 128], BF16))   # uninit warm-up weights
        DUMS = e(nc.sbuf_tensor([128, L], BF16))     # uninit warm-up stream
        SCR = e(nc.sbuf_tensor([128, 8], F32))       # dummy act in
        SCR2 = e(nc.sbuf_tensor([128, 8], F32))      # dummy act out
        PSUMS = e(nc.psum_tensor([128, 2 * L], F32))
        PSUMR = e(nc.psum_tensor([128, 6, L], F32))
        s_q = e(nc.semaphore("s_q"))
        s_k = e(nc.semaphore("s_k"))
        s_dg = e(nc.semaphore("s_dg"))
        s_am = e(nc.semaphore("s_am"))
        s_vh = e(nc.semaphore("s_vh"))
        s_t0 = e(nc.semaphore("s_t0"))
        s_n = e(nc.semaphore("s_n"))
        s_red = e(nc.semaphore("s_red"))
        s_act = e(nc.semaphore("s_act"))
        s_ampv = e(nc.semaphore("s_ampv"))
        s_scores = e(nc.semaphore("s_scores"))
        s_exp = e(nc.semaphore("s_exp"))
        s_av = e(nc.semaphore("s_av"))
        s_rcp = e(nc.semaphore("s_rcp"))
        s_norm = e(nc.semaphore("s_norm"))
        block = e(nc.Block())

        XHL = [QHL, KHL]

        # feature slots rotate 3-deep: unit g -> slots (2*(g%3), +1) of PSUMR.
        # Q: [sin|cos]=[lo|hi]; K: [cos|sin]=[lo|hi] so one merged act writes
        # FQRAW=[sin,cos] / FK=[cos,sin].
        def banks(g):
            rA = 2 * (g % 3)
            if g % 2 == 0:
                return rA, rA + 1  # sin, cos
            return rA + 1, rA      # sin, cos (cos occupies the lower slot)

        # AV output banks: ib0/ib1 -> PSUMS cols 0/L; ib2/ib3 -> PSUMR slots 0/1
        def av_out(ib):
            if ib < 2:
                return PSUMS[:, ib * L : ib * L + 65]
            return PSUMR[:, ib - 2, 0:65]

        def av_num(ib):
            if ib < 2:
                return PSUMS[:, ib * L : ib * L + D]
            return PSUMR[:, ib - 2, 0:D]

        def av_den(ib):
            if ib < 2:
                return PSUMS[:, ib * L + 64 : ib * L + 65]
            return PSUMR[:, ib - 2, 64:65]

        # score-chunk sources: jb0/jb1 inline in PSUMS; jb2/jb3 deferred in
        # PSUMR slots 4/5 (free right after act(5), the last slot-45 reader)
        def sc_bank(jb):
            if jb < 2:
                return PSUMS[:, jb * L : (jb + 1) * L]
            return PSUMR[:, 2 + jb, :]

        out_r = out_ext.rearrange("(g p) c -> p g c", p=128)

        @block.sync
        def _(sync):
            sync.dma_start(out=QHL[:], in_=qhl_ext[:]).then_inc(s_q, 16)
            sync.dma_start(out=KHL[:], in_=khl_ext[:]).then_inc(s_k, 16)
            for ib in (0, 2):
                sync.wait_ge(s_norm, ib + 1)
                sync.dma_start(
                    out=out_r[:, ib : ib + 1, :], in_=OUT[:, ib : ib + 1, :]
                ).then_inc(s_q, 16)

        @block.gpsimd
        def _(gpsimd):
            gpsimd.dma_start(out=DG[:, 0:2, :], in_=dg_ext[:, 0:2, :]).then_inc(s_dg, 16)
            gpsimd.dma_start(out=DG[:, 2:9, :], in_=dg_ext[:, 2:9, :]).then_inc(s_dg, 16)
            for ib in (1, 3):
                gpsimd.wait_ge(s_norm, ib + 1)
                gpsimd.dma_start(
                    out=out_r[:, ib : ib + 1, :], in_=OUT[:, ib : ib + 1, :]
                ).then_inc(s_vh, 16)

        @block.vector
        def _(vector):
            def rounds(g):
                # cos banks already hold t+0.25 (ones-row in DG cos slots), so
                # both banks take one identical magic-round: a single
                # 1024-wide 2D op over the unit's adjacent slot pair
                rA = 2 * (g % 3)
                vector.wait_ge(s_t0, 2 * g + 2)
                vector.tensor_scalar(
                    NR[:, g, :, :], PSUMR[:, rA : rA + 2, :], MAGIC, -MAGIC,
                    ALU.add, ALU.add,
                ).then_inc(s_n, 1)

            def amp(j):
                if j == 0:
                    vector.wait_ge(s_am, 16)
                vector.wait_ge(s_act, 2 * j + 1)
                vector.tensor_scalar_mul(
                    FQS[:, j], FQRAW[:, j], AMP[:, j : j + 1]
                ).then_inc(s_ampv, 1)

            rounds(0)
            rounds(1)
            rounds(2)
            amp(0)
            rounds(3)
            rounds(4)
            amp(1)
            rounds(5)
            rounds(6)
            amp(2)
            rounds(7)
            amp(3)
            for ib in range(4):
                vector.wait_ge(s_av, ib + 1)
                vector.reciprocal(RCP[:, ib : ib + 1], av_den(ib)).then_inc(
                    s_rcp, 1
                )

        @block.scalar
        def _(scalar):
            scalar.dma_start(out=AMP[:], in_=amp_ext[:]).then_inc(s_am, 16)
            scalar.dma_start(
                out=VH[:], in_=vh_ext.rearrange("(g p) c -> p g c", p=128)
            ).then_inc(s_vh, 16)
            # hoist the Sin table load off the critical path (no waits yet)
            scalar.activation(SCR2[:, 0:1], SCR[:, 0:1], AF.Sin)
            for g in range(8):
                j, s = g // 2, g % 2
                rA = 2 * (g % 3)
                scalar.wait_ge(s_red, 2 * (g + 1))
                dst = FQRAW[:, j] if s == 0 else FK[:, j]
                scalar.activation(
                    dst, PSUMR[:, rA : rA + 2, :], AF.Sin, scale=TWO_PI
                ).then_inc(s_act, 1)
            # hoist the Exp table load: runs right after the last Sin act
            scalar.activation(SCR2[:, 1:2], SCR[:, 1:2], AF.Exp)
            # jb0+jb1 complete back-to-back (inline final pass): one 1024-wide act
            scalar.wait_ge(s_scores, 2)
            scalar.activation(
                EXPT[:, 0:2, :], PSUMS[:, 0 : 2 * L], AF.Exp
            ).then_inc(s_exp, 2)
            for jb in (2, 3):
                scalar.wait_ge(s_scores, jb + 1)
                scalar.activation(
                    EXPT[:, jb], sc_bank(jb), AF.Exp
                ).then_inc(s_exp, 1)
            for ib in range(4):
                scalar.wait_ge(s_rcp, ib + 1)
                scalar.activation(
                    OUT[:, ib, :], av_num(ib), AF.Identity,
                    scale=RCP[:, ib : ib + 1],
                ).then_inc(s_norm, 1)

        @block.tensor
        def _(tensor):
            # HAM warm-up: keep the PE busy on garbage while DMAs land
            for _ in range(6):
                tensor.matmul(PSUMS[:, 0:L], DUMW[:], DUMS[:], start=True, stop=True)

            def t0(g):
                j, s = g // 2, g % 2
                sin_b, cos_b = banks(g)
                tensor.wait_ge(s_q if s == 0 else s_k, 16)
                if g == 0:
                    tensor.wait_ge(s_dg, 16)
                if g == 2:
                    tensor.wait_ge(s_dg, 32)
                if g >= 3:
                    tensor.wait_ge(s_act, g - 2)  # slots freed by act(g-3)
                tensor.matmul(PSUMR[:, sin_b, :], DG[:, 2 * j, :], XHL[s][:],
                              start=True, stop=False).then_inc(s_t0, 1)
                tensor.matmul(PSUMR[:, cos_b, :], DG[:, 2 * j + 1, :], XHL[s][:],
                              start=True, stop=False).then_inc(s_t0, 1)

            def red(g):
                # NR[g] holds rounds of [bank rA | bank rA+1]
                sin_b, cos_b = banks(g)
                rA = 2 * (g % 3)
                if g == 0:
                    tensor.wait_ge(s_dg, 32)  # -I lives in the second dg chunk
                tensor.wait_ge(s_n, g + 1)
                tensor.matmul(PSUMR[:, sin_b, :], DG[:, 8, :],
                              NR[:, g, sin_b - rA, :],
                              start=False, stop=True).then_inc(s_red, 1)
                tensor.matmul(PSUMR[:, cos_b, :], DG[:, 8, :],
                              NR[:, g, cos_b - rA, :],
                              start=False, stop=True).then_inc(s_red, 1)

            def score_mm(j, t, jb, start, stop, inc):
                mm = tensor.matmul(
                    sc_bank(jb),
                    FK[:, j, t, jb * 128 : (jb + 1) * 128],
                    FQS[:, j, t, :],
                    start=start,
                    stop=stop,
                )
                if inc:
                    mm.then_inc(s_scores, 1)

            def scores_inline(j):
                tensor.wait_ge(s_ampv, j + 1)
                tensor.wait_ge(s_act, 2 * j + 2)
                for t in range(2):
                    for jb in (0, 1):
                        score_mm(j, t, jb,
                                 start=(j == 0 and t == 0),
                                 stop=(j == 3 and t == 1),
                                 inc=(j == 3 and t == 1))

            def scores_defer(jb, js):
                for j in js:
                    for t in range(2):
                        score_mm(j, t, jb,
                                 start=(j == 0 and t == 0),
                                 stop=(j == 3 and t == 1),
                                 inc=(j == 3 and t == 1))

            def av_pass(jb):
                tensor.wait_ge(s_exp, max(2, jb + 1))
                for ib in range(4):
                    mm = tensor.matmul(
                        av_out(ib),
                        EXPT[:, jb, ib * 128 : (ib + 1) * 128],
                        VH[:, jb, :],
                        start=(jb == 0),
                        stop=(jb == 3),
                    )
                    if jb == 3:
                        mm.then_inc(s_av, 1)

            t0(0)
            t0(1)
            t0(2)
            red(0)
            red(1)
            for j in range(4):
                for g in (2 * j + 3, 2 * j + 4):
                    if 2 < g < 8:
                        t0(g)
                for g in (2 * j + 2, 2 * j + 3):
                    if g < 8:
                        red(g)
                if j < 3:
                    scores_inline(j)

            # deferred jb2/jb3 passes j=0..2 hide behind the last feature
            # rounds/acts (slots 4/5 free after act(5)); j=3 after the split acts
            tensor.wait_ge(s_act, 6)
            tensor.wait_ge(s_ampv, 3)
            scores_defer(2, (0, 1, 2))
            scores_defer(3, (0, 1, 2))
            scores_inline(3)
            scores_defer(2, (3,))
            scores_defer(3, (3,))
            tensor.wait_ge(s_vh, 16)
            av_pass(0)
            av_pass(1)
            av_pass(2)
            av_pass(3)

    return nc


def _get_nc():
    if "nc" not in _CACHE:
        _CACHE["nc"] = _build()
    return _CACHE["nc"]


def _make_consts():
    # slots: sin_j=2j, cos_j=2j+1 (row 127 = 0.25 -> t+0.25 via the ones row
    # that replaces lo[63] in qhl/khl), 8: -I
    dg = np.zeros((128, 9, 128), np.float32)
    amp = np.zeros((128, 4), np.float32)
    for j, (a, b) in enumerate(PAIRS):
        for sl in (2 * j, 2 * j + 1):
            for p in range(64):
                dg[p, sl, p] = W2PI[a]
                dg[p, sl, 64 + p] = W2PI[b]
                if 64 + p < 127:
                    dg[64 + p, sl, p] = W2PI[a]
                    dg[64 + p, sl, 64 + p] = W2PI[b]
        dg[127, 2 * j + 1, :] = 0.25
        amp[0:64, j] = C[a]
        amp[64:128, j] = C[b]
    for p in range(128):
        dg[p, 8, p] = -1.0
    return _bf(dg), amp


def _make_in_maps(q, k, v):
    dg, amp = _make_consts()
    in_maps = []
    for b in range(B):
        def hilo(x):
            xt = np.ascontiguousarray(x.T.astype(np.float32))      # [64, 512]
            h = _bf(xt)
            lo = _bf(xt - h.astype(np.float32))
            lo[63, :] = 1.0  # ones row: feeds the +0.25 in the DG cos slots
            return np.concatenate([h, lo], axis=0)                  # [128, 512]

        qhl = hilo(q[b])
        khl = hilo(k[b])
        vh = _bf(np.concatenate(
            [v[b].astype(np.float32), np.ones((L, 1), np.float32)], axis=1
        ))
        in_maps.append({"qhl": qhl, "khl": khl,
                        "vh": vh, "dg": dg, "amp": amp})
    return in_maps


def _run(in_maps, **kw):
    nc = _get_nc()
    return run_bass_kernel_spmd(nc, in_maps, core_ids=list(range(8)), **kw)


def kernel(q: np.ndarray, k: np.ndarray, v: np.ndarray) -> np.ndarray:
    res = _run(_make_in_maps(q, k, v))
    out = np.stack([res.results[b]["out"] for b in range(B)]).astype(np.float32)
    return out


# revision 5
# speedup vs baseline: 1.0063x; 1.0063x over previous
"""Additive attention kernel for 8 Trainium2 NeuronCores (v2).

Math: scores[b,i,j] = sum_d tanh(q[b,i,d] + k[b,j,d]); out = softmax_j(scores) @ v.
tanh(s) ~= sum_m C[m] sin(W[m] s) (M=8, bf16-exact W/2pi) and
sin(w(q+k)) = sin(wq)cos(wk) + cos(wq)sin(wk) -> rank-1024 bf16 PE matmul.

Angle path in turns (t = w x / 2pi):
  t0_psum = diag(w/2pi) @ (x_hi + x_lo)        (PE; host splits x = hi+lo bf16)
  n       = (t0 + MAGIC) - MAGIC               (DVE magic round; cos: +0.25 shifted)
  red    += (-I) @ n                           (PE accumulate)
  feat    = Sin(2pi * red) -> bf16             (ScalarE, 1024-wide merged acts)

v2 scheduling: PE-clock warm-up dummies during DMA wait; Sin/Exp act tables
hoisted off the critical path via dummy activations; inputs DMA'd on three
queues in parallel; K-side acts merged 1024-wide; tail pipelined per output
chunk with out-DMAs split across queues.

Sharding: B=8 -> one batch per core, no collectives.
"""

import math

import numpy as np
import ml_dtypes

import concourse.bass as bass
import concourse.mybir as mybir
from concourse.bass_utils import run_bass_kernel_spmd

F32 = mybir.dt.float32
BF16 = mybir.dt.bfloat16
AF = mybir.ActivationFunctionType
ALU = mybir.AluOpType

W0 = [0.273822509, 0.825679394, 1.38832881, 1.96485759,
      2.55624192, 3.16272728, 3.77941797, 4.47596827]

B, L, D, M = 8, 512, 64, 8
TWO_PI = 2.0 * math.pi
MAGIC = 12582912.0  # 1.5 * 2^23

PAIRS = [(0, 1), (2, 3), (4, 5), (6, 7)]


def _bf(x):
    return np.asarray(x).astype(ml_dtypes.bfloat16)


def _fit_consts():
    w2pi = _bf(np.array(W0, np.float32) / TWO_PI).astype(np.float64)
    w_eff = w2pi * TWO_PI
    S = 9.8
    sg = np.linspace(-S, S, 4001)
    wts = np.exp(-(sg**2) / 4) + 0.02
    A = np.sin(np.outer(sg, w_eff)) * np.sqrt(wts)[:, None]
    c, *_ = np.linalg.lstsq(A, np.tanh(sg) * np.sqrt(wts), rcond=None)
    return w2pi.astype(np.float32), c.astype(np.float32)


W2PI, C = _fit_consts()

_CACHE = {}


def _build():
    nc = bass.Bass()
    qhl_ext = nc.declare_dram_parameter("qhl", [128, L], BF16, isOutput=False)
    khl_ext = nc.declare_dram_parameter("khl", [128, L], BF16, isOutput=False)
    vh_ext = nc.declare_dram_parameter("vh", [L, 65], BF16, isOutput=False)
    dg_ext = nc.declare_dram_parameter("dg", [128, 9, 128], BF16, isOutput=False)
    amp_ext = nc.declare_dram_parameter("amp", [128, 4], F32, isOutput=False)
    out_ext = nc.declare_dram_parameter("out", [L, D], F32, isOutput=True)

    from contextlib import ExitStack

    with ExitStack() as ctx:
        e = ctx.enter_context
        QHL = e(nc.sbuf_tensor("QHL", [128, L], BF16))
        KHL = e(nc.sbuf_tensor("KHL", [128, L], BF16))
        # DG slots: sin_j=2j, cos_j=2j+1 (row 127 = 0.25 ones -> t+0.25), 8: -I
        DG = e(nc.sbuf_tensor([128, 9, 128], BF16))
        AMP = e(nc.sbuf_tensor([128, 4], F32))
        VH = e(nc.sbuf_tensor([128, 4, 65], BF16))
        # per-unit round output: [unit, bank-within-slot-pair, L]
        NR = e(nc.sbuf_tensor("NR", [128, 8, 2, L], BF16))
        FQRAW = e(nc.sbuf_tensor([128, 4, 2, L], BF16))
        FQS = e(nc.sbuf_tensor([128, 4, 2, L], BF16))
        FK = e(nc.sbuf_tensor([128, 4, 2, L], BF16))
        EXPT = e(nc.sbuf_tensor([128, 4, L], BF16))
        RCP = e(nc.sbuf_tensor([128, 4], F32))
        OUT = e(nc.sbuf_tensor([128, 4, D], F32))
        DUMW = e(nc.sbuf_tensor([128, 128], BF16))   # uninit warm-up weights
        DUMS = e(nc.sbuf_tensor([128, L], BF16))     # uninit warm-up stream
        SCR = e(nc.sbuf_tensor([128, 8], F32))       # dummy act in
        SCR2 = e(nc.sbuf_tensor([128, 8], F32))      # dummy act out
        PSUMS = e(nc.psum_tensor([128, 2 * L], F32))
        PSUMR = e(nc.psum_tensor([128, 6, L], F32))
        s_q = e(nc.semaphore("s_q"))
        s_k = e(nc.semaphore("s_k"))
        s_dg = e(nc.semaphore("s_dg"))
        s_am = e(nc.semaphore("s_am"))
        s_vh = e(nc.semaphore("s_vh"))
        s_t0 = e(nc.semaphore("s_t0"))
        s_n = e(nc.semaphore("s_n"))
        s_red = e(nc.semaphore("s_red"))
        s_act = e(nc.semaphore("s_act"))
        s_ampv = e(nc.semaphore("s_ampv"))
        s_scores = e(nc.semaphore("s_scores"))
        s_exp = e(nc.semaphore("s_exp"))
        s_av = e(nc.semaphore("s_av"))
        s_rcp = e(nc.semaphore("s_rcp"))
        s_norm = e(nc.semaphore("s_norm"))
        block = e(nc.Block())

        XHL = [QHL, KHL]

        # feature slots rotate 3-deep: unit g -> slots (2*(g%3), +1) of PSUMR.
        # Q: [sin|cos]=[lo|hi]; K: [cos|sin]=[lo|hi] so one merged act writes
        # FQRAW=[sin,cos] / FK=[cos,sin].
        def banks(g):
            rA = 2 * (g % 3)
            if g % 2 == 0:
                return rA, rA + 1  # sin, cos
            return rA + 1, rA      # sin, cos (cos occupies the lower slot)

        # AV output banks: ib0/ib1 -> PSUMS cols 0/L; ib2/ib3 -> PSUMR slots 0/1
        def av_out(ib):
            if ib < 2:
                return PSUMS[:, ib * L : ib * L + 65]
            return PSUMR[:, ib - 2, 0:65]

        def av_num(ib):
            if ib < 2:
                return PSUMS[:, ib * L : ib * L + D]
            return PSUMR[:, ib - 2, 0:D]

        def av_den(ib):
            if ib < 2:
                return PSUMS[:, ib * L + 64 : ib * L + 65]
            return PSUMR[:, ib - 2, 64:65]

        # score-chunk sources: jb0/jb1 inline in PSUMS; jb2/jb3 deferred in
        # PSUMR slots 4/5 (free right after act(5), the last slot-45 reader)
        def sc_bank(jb):
            if jb < 2:
                return PSUMS[:, jb * L : (jb + 1) * L]
            return PSUMR[:, 2 + jb, :]

        out_r = out_ext.rearrange("(g p) c -> p g c", p=128)

        @block.sync
        def _(sync):
            sync.dma_start(out=QHL[:], in_=qhl_ext[:]).then_inc(s_q, 16)
            sync.dma_start(out=KHL[:], in_=khl_ext[:]).then_inc(s_k, 16)
            for ib in (0, 2):
                sync.wait_ge(s_norm, ib + 1)
                sync.dma_start(
                    out=out_r[:, ib : ib + 1, :], in_=OUT[:, ib : ib + 1, :]
                ).then_inc(s_q, 16)

        @block.gpsimd
        def _(gpsimd):
            gpsimd.dma_start(out=DG[:, 0:2, :], in_=dg_ext[:, 0:2, :]).then_inc(s_dg, 16)
            gpsimd.dma_start(out=DG[:, 2:9, :], in_=dg_ext[:, 2:9, :]).then_inc(s_dg, 16)
            gpsimd.wait_ge(s_norm, 2)
            gpsimd.dma_start(
                out=out_r[:, 1:2, :], in_=OUT[:, 1:2, :]
            ).then_inc(s_vh, 16)

        @block.vector
        def _(vector):
            def rounds(g):
                # cos banks already hold t+0.25 (ones-row in DG cos slots), so
                # both banks take one identical magic-round: a single
                # 1024-wide 2D op over the unit's adjacent slot pair
                rA = 2 * (g % 3)
                vector.wait_ge(s_t0, 2 * g + 2)
                vector.tensor_scalar(
                    NR[:, g, :, :], PSUMR[:, rA : rA + 2, :], MAGIC, -MAGIC,
                    ALU.add, ALU.add,
                ).then_inc(s_n, 1)

            def amp(j):
                if j == 0:
                    vector.wait_ge(s_am, 16)
                vector.wait_ge(s_act, 2 * j + 1)
                vector.tensor_scalar_mul(
                    FQS[:, j], FQRAW[:, j], AMP[:, j : j + 1]
                ).then_inc(s_ampv, 1)

            rounds(0)
            rounds(1)
            rounds(2)
            amp(0)
            rounds(3)
            rounds(4)
            amp(1)
            rounds(5)
            rounds(6)
            amp(2)
            rounds(7)
            amp(3)
            for ib in range(4):
                vector.wait_ge(s_av, ib + 1)
                vector.reciprocal(RCP[:, ib : ib + 1], av_den(ib)).then_inc(
                    s_rcp, 1
                )

        @block.scalar
        def _(scalar):
            scalar.dma_start(out=AMP[:], in_=amp_ext[:]).then_inc(s_am, 16)
            scalar.dma_start(
                out=VH[:], in_=vh_ext.rearrange("(g p) c -> p g c", p=128)
            ).then_inc(s_vh, 16)
            # hoist the Sin table load off the critical path (no waits yet)
            scalar.activation(SCR2[:, 0:1], SCR[:, 0:1], AF.Sin)
            for g in range(8):
                j, s = g // 2, g % 2
                rA = 2 * (g % 3)
                scalar.wait_ge(s_red, 2 * (g + 1))
                dst = FQRAW[:, j] if s == 0 else FK[:, j]
                scalar.activation(
                    dst, PSUMR[:, rA : rA + 2, :], AF.Sin, scale=TWO_PI
                ).then_inc(s_act, 1)
            # hoist the Exp table load: runs right after the last Sin act
            scalar.activation(SCR2[:, 1:2], SCR[:, 1:2], AF.Exp)
            # jb0+jb1 complete back-to-back (inline final pass): one 1024-wide act
            scalar.wait_ge(s_scores, 2)
            scalar.activation(
                EXPT[:, 0:2, :], PSUMS[:, 0 : 2 * L], AF.Exp
            ).then_inc(s_exp, 2)
            for jb in (2, 3):
                scalar.wait_ge(s_scores, jb + 1)
                scalar.activation(
                    EXPT[:, jb], sc_bank(jb), AF.Exp
                ).then_inc(s_exp, 1)
            for ib in range(4):
                scalar.wait_ge(s_rcp, ib + 1)
                scalar.activation(
                    OUT[:, ib, :], av_num(ib), AF.Identity,
                    scale=RCP[:, ib : ib + 1],
                ).then_inc(s_norm, 1)
            scalar.wait_ge(s_norm, 4)
            scalar.dma_start(
                out=out_r[:, 3:4, :], in_=OUT[:, 3:4, :]
            ).then_inc(s_am, 16)

        @block.tensor
        def _(tensor):
            # HAM warm-up: keep the PE busy on garbage while DMAs land
            for _ in range(6):
                tensor.matmul(PSUMS[:, 0:L], DUMW[:], DUMS[:], start=True, stop=True)

            def t0(g):
                j, s = g // 2, g % 2
                sin_b, cos_b = banks(g)
                tensor.wait_ge(s_q if s == 0 else s_k, 16)
                if g == 0:
                    tensor.wait_ge(s_dg, 16)
                if g == 2:
                    tensor.wait_ge(s_dg, 32)
                if g >= 3:
                    tensor.wait_ge(s_act, g - 2)  # slots freed by act(g-3)
                tensor.matmul(PSUMR[:, sin_b, :], DG[:, 2 * j, :], XHL[s][:],
                              start=True, stop=False).then_inc(s_t0, 1)
                tensor.matmul(PSUMR[:, cos_b, :], DG[:, 2 * j + 1, :], XHL[s][:],
                              start=True, stop=False).then_inc(s_t0, 1)

            def red(g):
                # NR[g] holds rounds of [bank rA | bank rA+1]
                sin_b, cos_b = banks(g)
                rA = 2 * (g % 3)
                if g == 0:
                    tensor.wait_ge(s_dg, 32)  # -I lives in the second dg chunk
                tensor.wait_ge(s_n, g + 1)
                tensor.matmul(PSUMR[:, sin_b, :], DG[:, 8, :],
                              NR[:, g, sin_b - rA, :],
                              start=False, stop=True).then_inc(s_red, 1)
                tensor.matmul(PSUMR[:, cos_b, :], DG[:, 8, :],
                              NR[:, g, cos_b - rA, :],
                              start=False, stop=True).then_inc(s_red, 1)

            def score_mm(j, t, jb, start, stop, inc):
                mm = tensor.matmul(
                    sc_bank(jb),
                    FK[:, j, t, jb * 128 : (jb + 1) * 128],
                    FQS[:, j, t, :],
                    start=start,
                    stop=stop,
                )
                if inc:
                    mm.then_inc(s_scores, 1)

            def scores_inline(j):
                tensor.wait_ge(s_ampv, j + 1)
                tensor.wait_ge(s_act, 2 * j + 2)
                for t in range(2):
                    for jb in (0, 1):
                        score_mm(j, t, jb,
                                 start=(j == 0 and t == 0),
                                 stop=(j == 3 and t == 1),
                                 inc=(j == 3 and t == 1))

            def scores_defer(jb, js):
                for j in js:
                    for t in range(2):
                        score_mm(j, t, jb,
                                 start=(j == 0 and t == 0),
                                 stop=(j == 3 and t == 1),
                                 inc=(j == 3 and t == 1))

            def av_pass(jb):
                tensor.wait_ge(s_exp, max(2, jb + 1))
                for ib in range(4):
                    mm = tensor.matmul(
                        av_out(ib),
                        EXPT[:, jb, ib * 128 : (ib + 1) * 128],
                        VH[:, jb, :],
                        start=(jb == 0),
                        stop=(jb == 3),
                    )
                    if jb == 3:
                        mm.then_inc(s_av, 1)

            t0(0)
            t0(1)
            t0(2)
            red(0)
            red(1)
            for j in range(4):
                for g in (2 * j + 3, 2 * j + 4):
                    if 2 < g < 8:
                        t0(g)
                for g in (2 * j + 2, 2 * j + 3):
                    if g < 8:
                        red(g)
                if j < 3:
                    scores_inline(j)

            # deferred jb2/jb3 passes j=0..2 hide behind the last feature
            # rounds/acts (slots 4/5 free after act(5)); j=3 after the split acts
            tensor.wait_ge(s_act, 6)
            tensor.wait_ge(s_ampv, 3)
            scores_defer(2, (0, 1, 2))
            scores_defer(3, (0, 1, 2))
            scores_inline(3)
            scores_defer(2, (3,))
            scores_defer(3, (3,))
            tensor.wait_ge(s_vh, 16)
            av_pass(0)
            av_pass(1)
            av_pass(2)
            av_pass(3)

    return nc


def _get_nc():
    if "nc" not in _CACHE:
        _CACHE["nc"] = _build()
    return _CACHE["nc"]


def _make_consts():
    # slots: sin_j=2j, cos_j=2j+1 (row 127 = 0.25 -> t+0.25 via the ones row
    # that replaces lo[63] in qhl/khl), 8: -I
    dg = np.zeros((128, 9, 128), np.float32)
    amp = np.zeros((128, 4), np.float32)
    for j, (a, b) in enumerate(PAIRS):
        for sl in (2 * j, 2 * j + 1):
            for p in range(64):
                dg[p, sl, p] = W2PI[a]
                dg[p, sl, 64 + p] = W2PI[b]
                if 64 + p < 127:
                    dg[64 + p, sl, p] = W2PI[a]
                    dg[64 + p, sl, 64 + p] = W2PI[b]
        dg[127, 2 * j + 1, :] = 0.25
        amp[0:64, j] = C[a]
        amp[64:128, j] = C[b]
    for p in range(128):
        dg[p, 8, p] = -1.0
    return _bf(dg), amp


def _make_in_maps(q, k, v):
    dg, amp = _make_consts()
    in_maps = []
    for b in range(B):
        def hilo(x):
            xt = np.ascontiguousarray(x.T.astype(np.float32))      # [64, 512]
            h = _bf(xt)
            lo = _bf(xt - h.astype(np.float32))
            lo[63, :] = 1.0  # ones row: feeds the +0.25 in the DG cos slots
            return np.concatenate([h, lo], axis=0)                  # [128, 512]

        qhl = hilo(q[b])
        khl = hilo(k[b])
        vh = _bf(np.concatenate(
            [v[b].astype(np.float32), np.ones((L, 1), np.float32)], axis=1
        ))
        in_maps.append({"qhl": qhl, "khl": khl,
                        "vh": vh, "dg": dg, "amp": amp})
    return in_maps


def _run(in_maps, **kw):
    nc = _get_nc()
    return run_bass_kernel_spmd(nc, in_maps, core_ids=list(range(8)), **kw)


def kernel(q: np.ndarray, k: np.ndarray, v: np.ndarray) -> np.ndarray:
    res = _run(_make_in_maps(q, k, v))
    out = np.stack([res.results[b]["out"] for b in range(B)]).astype(np.float32)
    return out


# revision 6
# speedup vs baseline: 1.0174x; 1.0110x over previous
"""Additive attention kernel for 8 Trainium2 NeuronCores (v2).

Math: scores[b,i,j] = sum_d tanh(q[b,i,d] + k[b,j,d]); out = softmax_j(scores) @ v.
tanh(s) ~= sum_m C[m] sin(W[m] s) (M=8, bf16-exact W/2pi) and
sin(w(q+k)) = sin(wq)cos(wk) + cos(wq)sin(wk) -> rank-1024 bf16 PE matmul.

Angle path in turns (t = w x / 2pi):
  t0_psum = diag(w/2pi) @ (x_hi + x_lo)        (PE; host splits x = hi+lo bf16)
  n       = (t0 + MAGIC) - MAGIC               (DVE magic round; cos: +0.25 shifted)
  red    += (-I) @ n                           (PE accumulate)
  feat    = Sin(2pi * red) -> bf16             (ScalarE, 1024-wide merged acts)

v2 scheduling: PE-clock warm-up dummies during DMA wait; Sin/Exp act tables
hoisted off the critical path via dummy activations; inputs DMA'd on three
queues in parallel; K-side acts merged 1024-wide; tail pipelined per output
chunk with out-DMAs split across queues.

Sharding: B=8 -> one batch per core, no collectives.
"""

import math

import numpy as np
import ml_dtypes

import concourse.bass as bass
import concourse.mybir as mybir
from concourse.bass_utils import run_bass_kernel_spmd

F32 = mybir.dt.float32
BF16 = mybir.dt.bfloat16
AF = mybir.ActivationFunctionType
ALU = mybir.AluOpType

W0 = [0.273822509, 0.825679394, 1.38832881, 1.96485759,
      2.55624192, 3.16272728, 3.77941797, 4.47596827]

B, L, D, M = 8, 512, 64, 8
TWO_PI = 2.0 * math.pi
MAGIC = 12582912.0  # 1.5 * 2^23

PAIRS = [(0, 1), (2, 3), (4, 5), (6, 7)]


def _bf(x):
    return np.asarray(x).astype(ml_dtypes.bfloat16)


def _fit_consts():
    w2pi = _bf(np.array(W0, np.float32) / TWO_PI).astype(np.float64)
    w_eff = w2pi * TWO_PI
    S = 9.8
    sg = np.linspace(-S, S, 4001)
    wts = np.exp(-(sg**2) / 4) + 0.02
    A = np.sin(np.outer(sg, w_eff)) * np.sqrt(wts)[:, None]
    c, *_ = np.linalg.lstsq(A, np.tanh(sg) * np.sqrt(wts), rcond=None)
    return w2pi.astype(np.float32), c.astype(np.float32)


W2PI, C = _fit_consts()

_CACHE = {}


def _build():
    nc = bass.Bass()
    qhl_ext = nc.declare_dram_parameter("qhl", [128, L], BF16, isOutput=False)
    khl_ext = nc.declare_dram_parameter("khl", [128, L], BF16, isOutput=False)
    vh_ext = nc.declare_dram_parameter("vh", [L, 65], BF16, isOutput=False)
    dg_ext = nc.declare_dram_parameter("dg", [128, 9, 128], BF16, isOutput=False)
    amp_ext = nc.declare_dram_parameter("amp", [128, 4], F32, isOutput=False)
    out_ext = nc.declare_dram_parameter("out", [L, D], F32, isOutput=True)

    from contextlib import ExitStack

    with ExitStack() as ctx:
        e = ctx.enter_context
        QHL = e(nc.sbuf_tensor("QHL", [128, L], BF16))
        KHL = e(nc.sbuf_tensor("KHL", [128, L], BF16))
        # DG slots: sin_j=2j, cos_j=2j+1 (row 127 = 0.25 ones -> t+0.25), 8: -I
        DG = e(nc.sbuf_tensor([128, 9, 128], BF16))
        AMP = e(nc.sbuf_tensor([128, 4], F32))
        VH = e(nc.sbuf_tensor([128, 4, 65], BF16))
        # per-unit round output: [unit, bank-within-slot-pair, L]
        NR = e(nc.sbuf_tensor("NR", [128, 8, 2, L], BF16))
        FQRAW = e(nc.sbuf_tensor([128, 4, 2, L], BF16))
        FQS = e(nc.sbuf_tensor([128, 4, 2, L], BF16))
        FK = e(nc.sbuf_tensor([128, 4, 2, L], BF16))
        EXPT = e(nc.sbuf_tensor([128, 4, L], BF16))
        RCP = e(nc.sbuf_tensor([128, 4], F32))
        OUT = e(nc.sbuf_tensor([128, 4, D], F32))
        DUMW = e(nc.sbuf_tensor([128, 128], BF16))   # uninit warm-up weights
        DUMS = e(nc.sbuf_tensor([128, L], BF16))     # uninit warm-up stream
        SCR = e(nc.sbuf_tensor([128, 8], F32))       # dummy act in
        SCR2 = e(nc.sbuf_tensor([128, 8], F32))      # dummy act out
        PSUMS = e(nc.psum_tensor([128, 2 * L], F32))
        PSUMR = e(nc.psum_tensor([128, 6, L], F32))
        s_q = e(nc.semaphore("s_q"))
        s_k = e(nc.semaphore("s_k"))
        s_dg = e(nc.semaphore("s_dg"))
        s_am = e(nc.semaphore("s_am"))
        s_vh = e(nc.semaphore("s_vh"))
        s_t0 = e(nc.semaphore("s_t0"))
        s_n = e(nc.semaphore("s_n"))
        s_red = e(nc.semaphore("s_red"))
        s_act = e(nc.semaphore("s_act"))
        s_ampv = e(nc.semaphore("s_ampv"))
        s_scores = e(nc.semaphore("s_scores"))
        s_exp = e(nc.semaphore("s_exp"))
        s_av = e(nc.semaphore("s_av"))
        s_rcp = e(nc.semaphore("s_rcp"))
        s_norm = e(nc.semaphore("s_norm"))
        block = e(nc.Block())

        XHL = [QHL, KHL]

        # feature slots rotate 3-deep: unit g -> slots (2*(g%3), +1) of PSUMR.
        # Q: [sin|cos]=[lo|hi]; K: [cos|sin]=[lo|hi] so one merged act writes
        # FQRAW=[sin,cos] / FK=[cos,sin].
        def banks(g):
            rA = 2 * (g % 3)
            if g % 2 == 0:
                return rA, rA + 1  # sin, cos
            return rA + 1, rA      # sin, cos (cos occupies the lower slot)

        # AV output banks: ib0/ib1 -> PSUMS cols 0/L; ib2/ib3 -> PSUMR slots 0/1
        def av_out(ib):
            if ib < 2:
                return PSUMS[:, ib * L : ib * L + 65]
            return PSUMR[:, ib - 2, 0:65]

        def av_num(ib):
            if ib < 2:
                return PSUMS[:, ib * L : ib * L + D]
            return PSUMR[:, ib - 2, 0:D]

        def av_den(ib):
            if ib < 2:
                return PSUMS[:, ib * L + 64 : ib * L + 65]
            return PSUMR[:, ib - 2, 64:65]

        # score-chunk sources: jb0/jb1 inline in PSUMS; jb2/jb3 deferred in
        # PSUMR slots 4/5 (free right after act(5), the last slot-45 reader)
        def sc_bank(jb):
            if jb < 2:
                return PSUMS[:, jb * L : (jb + 1) * L]
            return PSUMR[:, 2 + jb, :]

        out_r = out_ext.rearrange("(g p) c -> p g c", p=128)

        @block.sync
        def _(sync):
            sync.dma_start(out=QHL[:], in_=qhl_ext[:]).then_inc(s_q, 16)
            sync.dma_start(out=KHL[:], in_=khl_ext[:]).then_inc(s_k, 16)
            for ib in (0, 2):
                sync.wait_ge(s_norm, ib + 1)
                sync.dma_start(
                    out=out_r[:, ib : ib + 1, :], in_=OUT[:, ib : ib + 1, :]
                ).then_inc(s_q, 16)

        @block.gpsimd
        def _(gpsimd):
            gpsimd.dma_start(out=DG[:, 0:2, :], in_=dg_ext[:, 0:2, :]).then_inc(s_dg, 16)
            gpsimd.dma_start(out=DG[:, 2:9, :], in_=dg_ext[:, 2:9, :]).then_inc(s_dg, 16)
            gpsimd.wait_ge(s_norm, 2)
            gpsimd.dma_start(
                out=out_r[:, 1:2, :], in_=OUT[:, 1:2, :]
            ).then_inc(s_vh, 16)

        @block.vector
        def _(vector):
            def rounds(g):
                # cos banks already hold t+0.25 (ones-row in DG cos slots), so
                # both banks take one identical magic-round: a single
                # 1024-wide 2D op over the unit's adjacent slot pair
                rA = 2 * (g % 3)
                vector.wait_ge(s_t0, 2 * g + 2)
                vector.tensor_scalar(
                    NR[:, g, :, :], PSUMR[:, rA : rA + 2, :], MAGIC, -MAGIC,
                    ALU.add, ALU.add,
                ).then_inc(s_n, 1)

            def amp(j):
                if j == 0:
                    vector.wait_ge(s_am, 16)
                vector.wait_ge(s_act, 2 * j + 1)
                vector.tensor_scalar_mul(
                    FQS[:, j], FQRAW[:, j], AMP[:, j : j + 1]
                ).then_inc(s_ampv, 1)

            rounds(0)
            rounds(1)
            rounds(2)
            amp(0)
            rounds(3)
            rounds(4)
            rounds(5)
            amp(1)
            rounds(6)
            rounds(7)
            amp(2)
            amp(3)
            for ib in range(4):
                vector.wait_ge(s_av, ib + 1)
                vector.reciprocal(RCP[:, ib : ib + 1], av_den(ib)).then_inc(
                    s_rcp, 1
                )

        @block.scalar
        def _(scalar):
            scalar.dma_start(out=AMP[:], in_=amp_ext[:]).then_inc(s_am, 16)
            scalar.dma_start(
                out=VH[:], in_=vh_ext.rearrange("(g p) c -> p g c", p=128)
            ).then_inc(s_vh, 16)
            # hoist the Sin table load off the critical path (no waits yet)
            scalar.activation(SCR2[:, 0:1], SCR[:, 0:1], AF.Sin)
            for g in range(8):
                j, s = g // 2, g % 2
                rA = 2 * (g % 3)
                scalar.wait_ge(s_red, 2 * (g + 1))
                dst = FQRAW[:, j] if s == 0 else FK[:, j]
                scalar.activation(
                    dst, PSUMR[:, rA : rA + 2, :], AF.Sin, scale=TWO_PI
                ).then_inc(s_act, 1)
            # hoist the Exp table load: runs right after the last Sin act
            scalar.activation(SCR2[:, 1:2], SCR[:, 1:2], AF.Exp)
            # jb0+jb1 complete back-to-back (inline final pass): one 1024-wide act
            scalar.wait_ge(s_scores, 2)
            scalar.activation(
                EXPT[:, 0:2, :], PSUMS[:, 0 : 2 * L], AF.Exp
            ).then_inc(s_exp, 2)
            for jb in (2, 3):
                scalar.wait_ge(s_scores, jb + 1)
                scalar.activation(
                    EXPT[:, jb], sc_bank(jb), AF.Exp
                ).then_inc(s_exp, 1)
            for ib in range(4):
                scalar.wait_ge(s_rcp, ib + 1)
                scalar.activation(
                    OUT[:, ib, :], av_num(ib), AF.Identity,
                    scale=RCP[:, ib : ib + 1],
                ).then_inc(s_norm, 1)
            scalar.wait_ge(s_norm, 4)
            scalar.dma_start(
                out=out_r[:, 3:4, :], in_=OUT[:, 3:4, :]
            ).then_inc(s_am, 16)

        @block.tensor
        def _(tensor):
            # HAM warm-up: keep the PE busy on garbage while DMAs land
            for _ in range(6):
                tensor.matmul(PSUMS[:, 0:L], DUMW[:], DUMS[:], start=True, stop=True)

            def t0(g):
                j, s = g // 2, g % 2
                sin_b, cos_b = banks(g)
                tensor.wait_ge(s_q if s == 0 else s_k, 16)
                if g == 0:
                    tensor.wait_ge(s_dg, 16)
                if g == 2:
                    tensor.wait_ge(s_dg, 32)
                if g >= 3:
                    tensor.wait_ge(s_act, g - 2)  # slots freed by act(g-3)
                tensor.matmul(PSUMR[:, sin_b, :], DG[:, 2 * j, :], XHL[s][:],
                              start=True, stop=False).then_inc(s_t0, 1)
                tensor.matmul(PSUMR[:, cos_b, :], DG[:, 2 * j + 1, :], XHL[s][:],
                              start=True, stop=False).then_inc(s_t0, 1)

            def red(g):
                # NR[g] holds rounds of [bank rA | bank rA+1]
                sin_b, cos_b = banks(g)
                rA = 2 * (g % 3)
                if g == 0:
                    tensor.wait_ge(s_dg, 32)  # -I lives in the second dg chunk
                tensor.wait_ge(s_n, g + 1)
                tensor.matmul(PSUMR[:, sin_b, :], DG[:, 8, :],
                              NR[:, g, sin_b - rA, :],
                              start=False, stop=True).then_inc(s_red, 1)
                tensor.matmul(PSUMR[:, cos_b, :], DG[:, 8, :],
                              NR[:, g, cos_b - rA, :],
                              start=False, stop=True).then_inc(s_red, 1)

            def score_mm(j, t, jb, start, stop, inc):
                mm = tensor.matmul(
                    sc_bank(jb),
                    FK[:, j, t, jb * 128 : (jb + 1) * 128],
                    FQS[:, j, t, :],
                    start=start,
                    stop=stop,
                )
                if inc:
                    mm.then_inc(s_scores, 1)

            def scores_inline(j):
                tensor.wait_ge(s_ampv, j + 1)
                tensor.wait_ge(s_act, 2 * j + 2)
                for t in range(2):
                    for jb in (0, 1):
                        score_mm(j, t, jb,
                                 start=(j == 0 and t == 0),
                                 stop=(j == 3 and t == 1),
                                 inc=(j == 3 and t == 1))

            def scores_defer(jb, js):
                for j in js:
                    for t in range(2):
                        score_mm(j, t, jb,
                                 start=(j == 0 and t == 0),
                                 stop=(j == 3 and t == 1),
                                 inc=(j == 3 and t == 1))

            def av_pass(jb):
                tensor.wait_ge(s_exp, max(2, jb + 1))
                for ib in range(4):
                    mm = tensor.matmul(
                        av_out(ib),
                        EXPT[:, jb, ib * 128 : (ib + 1) * 128],
                        VH[:, jb, :],
                        start=(jb == 0),
                        stop=(jb == 3),
                    )
                    if jb == 3:
                        mm.then_inc(s_av, 1)

            t0(0)
            t0(1)
            t0(2)
            red(0)
            red(1)
            for j in range(4):
                for g in (2 * j + 3, 2 * j + 4):
                    if 2 < g < 8:
                        t0(g)
                for g in (2 * j + 2, 2 * j + 3):
                    if g < 8:
                        red(g)
                if j < 3:
                    scores_inline(j)

            # deferred jb2/jb3 passes j=0..2 hide behind the last feature
            # rounds/acts (slots 4/5 free after act(5)); j=3 after the split acts
            tensor.wait_ge(s_act, 6)
            tensor.wait_ge(s_ampv, 3)
            scores_defer(2, (0, 1, 2))
            scores_defer(3, (0, 1, 2))
            scores_inline(3)
            scores_defer(2, (3,))
            scores_defer(3, (3,))
            tensor.wait_ge(s_vh, 16)
            av_pass(0)
            av_pass(1)
            av_pass(2)
            av_pass(3)

    return nc


def _get_nc():
    if "nc" not in _CACHE:
        _CACHE["nc"] = _build()
    return _CACHE["nc"]


def _make_consts():
    # slots: sin_j=2j, cos_j=2j+1 (row 127 = 0.25 -> t+0.25 via the ones row
    # that replaces lo[63] in qhl/khl), 8: -I
    dg = np.zeros((128, 9, 128), np.float32)
    amp = np.zeros((128, 4), np.float32)
    for j, (a, b) in enumerate(PAIRS):
        for sl in (2 * j, 2 * j + 1):
            for p in range(64):
                dg[p, sl, p] = W2PI[a]
                dg[p, sl, 64 + p] = W2PI[b]
                if 64 + p < 127:
                    dg[64 + p, sl, p] = W2PI[a]
                    dg[64 + p, sl, 64 + p] = W2PI[b]
        dg[127, 2 * j + 1, :] = 0.25
        amp[0:64, j] = C[a]
        amp[64:128, j] = C[b]
    for p in range(128):
        dg[p, 8, p] = -1.0
    return _bf(dg), amp


def _make_in_maps(q, k, v):
    dg, amp = _make_consts()
    in_maps = []
    for b in range(B):
        def hilo(x):
            xt = np.ascontiguousarray(x.T.astype(np.float32))      # [64, 512]
            h = _bf(xt)
            lo = _bf(xt - h.astype(np.float32))
            lo[63, :] = 1.0  # ones row: feeds the +0.25 in the DG cos slots
            return np.concatenate([h, lo], axis=0)                  # [128, 512]

        qhl = hilo(q[b])
        khl = hilo(k[b])
        vh = _bf(np.concatenate(
            [v[b].astype(np.float32), np.ones((L, 1), np.float32)], axis=1
        ))
        in_maps.append({"qhl": qhl, "khl": khl,
                        "vh": vh, "dg": dg, "amp": amp})
    return in_maps


def _run(in_maps, **kw):
    nc = _get_nc()
    return run_bass_kernel_spmd(nc, in_maps, core_ids=list(range(8)), **kw)


def kernel(q: np.ndarray, k: np.ndarray, v: np.ndarray) -> np.ndarray:
    res = _run(_make_in_maps(q, k, v))
    out = np.stack([res.results[b]["out"] for b in range(B)]).astype(np.float32)
    return out
